# revision 1
# baseline (speedup 1.0000x reference)
"""NetVLAD on 8 Trainium2 NeuronCores — self-contained kernel.

Problem: x [32, 2048, 1024] f32, W [64, 1024] f32, centroids [64, 1024] f32
  -> out [32, 65536] f32  (NetVLAD pooling: per-frame L2 norm, soft-assign
  softmax over 64 clusters, residual aggregation, intra + global L2 norm).

Sharding: data-parallel over batch — 4 samples per core, W/centroids
replicated; no cross-core communication. Per-core program (all bf16 PE work):
  r[m] = 1/||x[m,:]||  (bf16 x; exp/ln chain — ACT Rsqrt is banned)
  z = x @ W^T          (contract D via xbar-DMA-transposed bf16 x tiles)
  a' = exp(r*z - ln(sum_k exp(r*z)) - ln ||x||)   (= softmax(r·z)·r)
  agg = a'^T @ x, colsum = a'^T @ ||x||  (= sum_m softmax)  [PE col-packed]
  vlad = agg - colsum*c; per-row L2 normalize; global L2 normalize.
"""

import json

import numpy as np

import concourse.bass as bass
import concourse.mybir as mybir
import concourse.tile as tile

F32 = mybir.dt.float32
BF16 = mybir.dt.bfloat16
AF = mybir.ActivationFunctionType
OP = mybir.AluOpType

B = 32
N_CORES = 8
B_PER_CORE = B // N_CORES
M = 2048
D = 1024
K = 64
NQ = 4           # quarters per sample
TQ = 4           # m-tiles per quarter

_PATCHED = False


def _split_waits_json(bir: dict, max_waits: int = 1) -> dict:
    """Split multi-wait sync infos into standalone EventSemaphore waits.

    The walrus build in this image supports a single sync-wait command per
    instruction, while Tile's sem assignment emits several (e.g. the
    kernel-tail Drain waits on every DMAHW lane). Hoisting the extra waits
    into preceding single-wait EventSemaphore instructions on the same
    engine is semantics-preserving for monotonic semaphores.
    """
    ctr = 0
    for f in bir.get("functions", []):
        for blk in f.get("blocks", []):
            insts = blk.get("instructions", [])
            new = []
            for inst in insts:
                si = inst.get("sync_info")
                waits = si.get("on_wait", []) if si else []
                if len(waits) > max_waits:
                    head, keep = waits[:-max_waits], waits[-max_waits:]
                    for w in head:
                        ctr += 1
                        new.append({
                            "debug": inst.get("debug", 0),
                            "engine": inst["engine"],
                            "ins": [],
                            "name": f"{inst['name']}-wsplit{ctr}",
                            "opcode": "EventSemaphore",
                            "outs": [],
                            "sync_info": {"on_update": [], "on_wait": [w]},
                        })
                    si["on_wait"] = keep
                new.append(inst)
            blk["instructions"] = new
    return bir


def _apply_patch():
    global _PATCHED
    if _PATCHED:
        return
    import concourse.bass_utils as bu
    import concourse.bass2jax as b2j
    orig = bu.compile_bir_kernel

    def patched(bir_json, tmpdir, neff_name="file.neff"):
        d = json.loads(bir_json)
        d = _split_waits_json(d, 1)
        return orig(json.dumps(d).encode(), tmpdir, neff_name)

    bu.compile_bir_kernel = patched
    b2j.compile_bir_kernel = patched
    _PATCHED = True


def build_nc():
    nc = bass.Bass()
    x = nc.dram_tensor("x", [B_PER_CORE, M, D], F32, kind="ExternalInput")
    W = nc.dram_tensor("W", [K, D], F32, kind="ExternalInput")
    C = nc.dram_tensor("centroids", [K, D], F32, kind="ExternalInput")
    out = nc.dram_tensor("out", [B_PER_CORE, K * D], F32, kind="ExternalOutput")
    ind2_d = nc.dram_tensor("ind2", [2, 128], F32, kind="ExternalInput")
    indK_d = nc.dram_tensor("indK", [128, 2], F32, kind="ExternalInput")

    xr = x[:, :, :].rearrange("s (q t p) d -> s q p t d", q=NQ, t=TQ, p=128)
    outr = out[:, :].rearrange("s (k d) -> s k d", d=D)

    from contextlib import ExitStack
    with tile.TileContext(nc) as tc, ExitStack() as es:
        singles = es.enter_context(tc.tile_pool(name="singles", bufs=1))
        xqpool = es.enter_context(tc.tile_pool(name="xqp", bufs=6))
        xTpool = es.enter_context(tc.tile_pool(name="xTp", bufs=4))
        sqpool = es.enter_context(tc.tile_pool(name="sqp", bufs=3))
        statpool = es.enter_context(tc.tile_pool(name="statp", bufs=4))
        epool = es.enter_context(tc.tile_pool(name="ep", bufs=3))
        apool = es.enter_context(tc.tile_pool(name="apl", bufs=4))
        rspool = es.enter_context(tc.tile_pool(name="rsp", bufs=4))
        tailpool = es.enter_context(tc.tile_pool(name="tailp", bufs=2))
        zpsum = es.enter_context(tc.tile_pool(name="zps", bufs=2, space="PSUM"))
        aggpsum = es.enter_context(
            tc.tile_pool(name="aggps", bufs=2, space="PSUM"))
        cspsum = es.enter_context(tc.tile_pool(name="csps", bufs=1, space="PSUM"))
        tailpsum = es.enter_context(tc.tile_pool(name="tps", bufs=1, space="PSUM"))

        Wbf = singles.tile([K, D], BF16)
        nc.gpsimd.dma_start(out=Wbf, in_=W[:, :])
        WT = singles.tile([128, 8, K], BF16)  # WT[q, c, k] = W[k, 128c+q]
        nc.sync.dma_start(out=WT, in_=Wbf, transpose=True)
        cpair = singles.tile([128, D], F32)
        nc.gpsimd.dma_start(out=cpair[0:64, :], in_=C[:, :])
        nc.gpsimd.dma_start(out=cpair[64:128, :], in_=C[:, :])
        ind2 = singles.tile([2, 128], F32)
        nc.sync.dma_start(out=ind2, in_=ind2_d[:, :])
        indK = singles.tile([128, 2], F32)
        nc.sync.dma_start(out=indK, in_=indK_d[:, :])

        def quarter(s, q, agg, cs):
            base = 64 * (s % 2)
            first = q == 0
            last = q == NQ - 1
            xq = xqpool.tile([128, TQ, D], BF16, tag="xq", name=f"xq_{s}_{q}")
            nc.gpsimd.dma_start(out=xq, in_=xr[s, q])
            ssq = statpool.tile([128, TQ], F32, tag="ssq", name=f"ssq_{s}_{q}")
            zq = zpsum.tile([128, TQ, K], F32, tag="zq", name=f"zq_{s}_{q}")
            for i in range(TQ):
                t = TQ * q + i
                sq = sqpool.tile([128, D], BF16, tag="sq", name=f"sq_{s}_{t}")
                if t % 8 < 3:
                    nc.scalar.activation(
                        out=sq, in_=xq[:, i, :], func=AF.Square,
                        accum_out=ssq[:, i:i + 1],
                    )
                else:
                    nc.vector.tensor_mul(sq, xq[:, i, :], xq[:, i, :])
                    nc.vector.tensor_scalar(
                        out=sq, in0=sq, scalar1=1.0, scalar2=0.0,
                        op0=OP.mult, op1=OP.add, accum_out=ssq[:, i:i + 1],
                    )
                xT = xTpool.tile([128, 8, 128], BF16, tag="xT",
                                 name=f"xT_{s}_{t}")
                nc.sync.dma_start(out=xT, in_=xq[:, i, :], transpose=True)
                for c in range(8):
                    nc.tensor.matmul(
                        zq[:, i, :], lhsT=xT[:, c, :], rhs=WT[:, c, :],
                        start=(c == 0), stop=(c == 7),
                    )
            lnt = statpool.tile([128, TQ], F32, tag="lnt", name=f"lnt_{s}_{q}")
            nc.scalar.activation(out=lnt, in_=ssq, func=AF.Ln)
            r = statpool.tile([128, TQ], F32, tag="r", name=f"r_{s}_{q}")
            nc.scalar.activation(out=r, in_=lnt, func=AF.Exp, scale=-0.5)
            invr = statpool.tile([128, TQ], BF16, tag="invr",
                                 name=f"invr_{s}_{q}")
            nc.scalar.activation(out=invr, in_=lnt, func=AF.Exp, scale=0.5)
            sden = statpool.tile([128, TQ], F32, tag="sden", name=f"sden_{s}_{q}")
            for i in range(TQ):
                e = epool.tile([128, K], BF16, tag="e", name=f"e_{s}_{q}_{i}")
                nc.scalar.activation(
                    out=e, in_=zq[:, i, :], func=AF.Exp, scale=r[:, i:i + 1]
                )
                nc.vector.reduce_sum(
                    out=sden[:, i:i + 1], in_=e, axis=mybir.AxisListType.X
                )
            lns = statpool.tile([128, TQ], F32, tag="lns", name=f"lns_{s}_{q}")
            nc.scalar.activation(out=lns, in_=sden, func=AF.Ln)
            h = statpool.tile([128, TQ], F32, tag="h", name=f"h_{s}_{q}")
            nc.vector.tensor_scalar(
                out=h, in0=lnt, scalar1=-0.5, scalar2=0.0,
                op0=OP.mult, op1=OP.add,
            )
            bias = statpool.tile([128, TQ], F32, tag="bias", name=f"bias_{s}_{q}")
            nc.vector.tensor_sub(bias, h, lns)
            for i in range(TQ):
                t = TQ * q + i
                a = apool.tile([128, K], BF16, tag="a", name=f"a_{s}_{t}")
                nc.scalar.activation(
                    out=a, in_=zq[:, i, :], func=AF.Exp,
                    scale=r[:, i:i + 1], bias=bias[:, i:i + 1],
                )
                st_ = first and i == 0
                sp_ = last and i == TQ - 1
                nc.tensor.matmul(
                    agg[base:base + 64, 0:512], lhsT=a, rhs=xq[:, i, 0:512],
                    start=st_, stop=sp_,
                )
                nc.tensor.matmul(
                    agg[base:base + 64, 512:1024], lhsT=a,
                    rhs=xq[:, i, 512:1024], start=st_, stop=sp_,
                )
                nc.tensor.matmul(
                    cs[base:base + 64, 0:1], lhsT=a, rhs=invr[:, i:i + 1],
                    start=st_, stop=sp_,
                )

        def tail_pair(p, agg, cs):
            sa, sb = 2 * p, 2 * p + 1
            cssb = rspool.tile([128, 1], F32, tag="cssb", name=f"cssb_{p}")
            nc.vector.tensor_copy(out=cssb, in_=cs[:, 0:1])
            tmp = tailpool.tile([128, D], F32, tag="tmp", name=f"tmp_{p}")
            nc.vector.tensor_scalar_mul(tmp, cpair, cssb)
            vlad = tailpool.tile([128, D], F32, tag="vlad", name=f"vlad_{p}")
            nc.vector.tensor_sub(vlad, agg[:, :], tmp)
            sq2 = sqpool.tile([128, D], BF16, tag="sq", name=f"sqt_{p}")
            vssq = rspool.tile([128, 1], F32, tag="vssq", name=f"vssq_{p}")
            nc.scalar.activation(out=sq2, in_=vlad, func=AF.Square,
                                 accum_out=vssq)
            lnv = rspool.tile([128, 1], F32, tag="lnv", name=f"lnv_{p}")
            nc.scalar.activation(out=lnv, in_=vssq, func=AF.Ln)
            rv = rspool.tile([128, 1], F32, tag="rv", name=f"rv_{p}")
            nc.scalar.activation(out=rv, in_=lnv, func=AF.Exp, scale=-0.5)
            ssqn = rspool.tile([128, 1], F32, tag="ssqn", name=f"ssqn_{p}")
            nc.vector.tensor_scalar(
                out=ssqn, in0=vssq, scalar1=rv, scalar2=rv,
                op0=OP.mult, op1=OP.mult,
            )
            gsum = tailpsum.tile([2, 2], F32, tag="tps", name=f"gsum_{p}")
            nc.tensor.matmul(gsum[:, 0:1], lhsT=indK, rhs=ssqn,
                             start=True, stop=True)
            lng = rspool.tile([2, 1], F32, tag="lng", name=f"lng_{p}")
            nc.scalar.activation(out=lng, in_=gsum[:, 0:1], func=AF.Ln)
            ginv = rspool.tile([2, 1], F32, tag="ginv", name=f"ginv_{p}")
            nc.scalar.activation(out=ginv, in_=lng, func=AF.Exp, scale=-0.5)
            gb = tailpsum.tile([128, 2], F32, tag="tps", name=f"gb_{p}")
            nc.tensor.matmul(gb[:, 0:1], lhsT=ind2, rhs=ginv,
                             start=True, stop=True)
            fs = rspool.tile([128, 1], F32, tag="fs", name=f"fs_{p}")
            nc.vector.tensor_mul(fs, rv, gb[:, 0:1])
            osb = tailpool.tile([128, D], F32, tag="osb", name=f"osb_{p}")
            nc.vector.tensor_scalar_mul(osb, vlad, fs)
            nc.sync.dma_start(out=outr[sa], in_=osb[0:64, :])
            nc.sync.dma_start(out=outr[sb], in_=osb[64:128, :])

        for p in range(2):
            agg = aggpsum.tile([128, D], F32, tag="agg", name=f"agg_{p}")
            cs = cspsum.tile([128, 8], F32, tag="cs", name=f"cs_{p}")
            for s in (2 * p, 2 * p + 1):
                for q in range(NQ):
                    quarter(s, q, agg, cs)
            tail_pair(p, agg, cs)

    return nc


_NC_CACHE = None


def kernel(**inputs: np.ndarray) -> np.ndarray:
    global _NC_CACHE
    _apply_patch()
    from concourse.bass_utils import run_bass_kernel_spmd

    x = np.ascontiguousarray(np.asarray(inputs["x"], dtype=np.float32))
    W = np.ascontiguousarray(np.asarray(inputs["W"], dtype=np.float32))
    cent = np.ascontiguousarray(
        np.asarray(inputs["centroids"], dtype=np.float32))

    ind2 = np.zeros((2, 128), dtype=np.float32)
    ind2[0, 0:64] = 1.0
    ind2[1, 64:128] = 1.0
    indK = np.zeros((128, 2), dtype=np.float32)
    indK[0:64, 0] = 1.0
    indK[64:128, 1] = 1.0

    if _NC_CACHE is None:
        _NC_CACHE = build_nc()
    nc = _NC_CACHE

    in_maps = [
        {
            "x": np.ascontiguousarray(
                x[B_PER_CORE * c:B_PER_CORE * (c + 1)]),
            "W": W,
            "centroids": cent,
            "ind2": ind2,
            "indK": indK,
        }
        for c in range(N_CORES)
    ]
    res = run_bass_kernel_spmd(nc, in_maps, core_ids=list(range(N_CORES)))
    return np.concatenate([r["out"] for r in res.results], axis=0)



# revision 3
# speedup vs baseline: 1.0056x; 1.0056x over previous
"""NetVLAD on 8 Trainium2 NeuronCores — self-contained kernel.

Problem: x [32, 2048, 1024] f32, W [64, 1024] f32, centroids [64, 1024] f32
  -> out [32, 65536] f32  (NetVLAD pooling: per-frame L2 norm, soft-assign
  softmax over 64 clusters, residual aggregation, intra + global L2 norm).

Sharding: data-parallel over batch — 4 samples per core, W/centroids
replicated; no cross-core communication. Per-core program (bf16 PE work):
  r[m] = 1/||x[m,:]||  (exp/ln chain — ACT Rsqrt is banned)
  zT = W^T-stationary @ x^T-moving  (x^T via quarter-sized xbar DMA
       transposes; W^T chunks are the only PE stationaries -> cheap LDW)
  z natural via small [64,512] xbar transpose of zT
  a' = exp(r*z) * (r / sum_k exp(r*z))   (= softmax(r.z)*r)
  agg += a'^T @ x_raw, cs += a'^T @ ||x||  (PE, col-packed sample pairs)
  vlad = agg - cs*c; per-row L2 normalize; global L2 normalize.
"""

import json

import numpy as np

import concourse.bass as bass
import concourse.mybir as mybir
import concourse.tile as tile

F32 = mybir.dt.float32
BF16 = mybir.dt.bfloat16
AF = mybir.ActivationFunctionType
OP = mybir.AluOpType

B = 32
N_CORES = 8
B_PER_CORE = B // N_CORES
M = 2048
D = 1024
K = 64
NQ = 4           # quarters per sample (512 m each)
TQ = 4           # m sub-tiles per quarter (128 m each)

_PATCHED = False


def _split_waits_json(bir: dict, max_waits: int = 1) -> dict:
    """Split multi-wait sync infos into standalone EventSemaphore waits.

    The walrus build in this image supports a single sync-wait command per
    instruction, while Tile's sem assignment emits several (e.g. the
    kernel-tail Drain waits on every DMAHW lane). Hoisting the extra waits
    into preceding single-wait EventSemaphore instructions on the same
    engine is semantics-preserving for monotonic semaphores.
    """
    ctr = 0
    for f in bir.get("functions", []):
        for blk in f.get("blocks", []):
            insts = blk.get("instructions", [])
            new = []
            for inst in insts:
                si = inst.get("sync_info")
                waits = si.get("on_wait", []) if si else []
                if len(waits) > max_waits:
                    head, keep = waits[:-max_waits], waits[-max_waits:]
                    for w in head:
                        ctr += 1
                        new.append({
                            "debug": inst.get("debug", 0),
                            "engine": inst["engine"],
                            "ins": [],
                            "name": f"{inst['name']}-wsplit{ctr}",
                            "opcode": "EventSemaphore",
                            "outs": [],
                            "sync_info": {"on_update": [], "on_wait": [w]},
                        })
                    si["on_wait"] = keep
                new.append(inst)
            blk["instructions"] = new
    return bir


def _apply_patch():
    global _PATCHED
    if _PATCHED:
        return
    import concourse.bass_utils as bu
    import concourse.bass2jax as b2j
    orig = bu.compile_bir_kernel

    def patched(bir_json, tmpdir, neff_name="file.neff"):
        d = json.loads(bir_json)
        d = _split_waits_json(d, 1)
        return orig(json.dumps(d).encode(), tmpdir, neff_name)

    bu.compile_bir_kernel = patched
    b2j.compile_bir_kernel = patched
    _PATCHED = True


def build_nc():
    nc = bass.Bass()
    x = nc.dram_tensor("x", [B_PER_CORE, M, D], F32, kind="ExternalInput")
    W = nc.dram_tensor("W", [K, D], F32, kind="ExternalInput")
    C = nc.dram_tensor("centroids", [K, D], F32, kind="ExternalInput")
    out = nc.dram_tensor("out", [B_PER_CORE, K * D], F32, kind="ExternalOutput")
    ind2_d = nc.dram_tensor("ind2", [2, 128], F32, kind="ExternalInput")
    indK_d = nc.dram_tensor("indK", [128, 2], F32, kind="ExternalInput")

    # m = q*512 + p*4 + t  -> 16KB contiguous HBM read per partition
    xr = x[:, :, :].rearrange("s (q p t) d -> s q p t d", q=NQ, p=128, t=TQ)
    outr = out[:, :].rearrange("s (k d) -> s k d", d=D)

    from contextlib import ExitStack
    with tile.TileContext(nc) as tc, ExitStack() as es:
        singles = es.enter_context(tc.tile_pool(name="singles", bufs=1))
        xqpool = es.enter_context(tc.tile_pool(name="xqp", bufs=4))
        xTpool = es.enter_context(tc.tile_pool(name="xTp", bufs=3))
        sqpool = es.enter_context(tc.tile_pool(name="sqp", bufs=2))
        zsbpool = es.enter_context(tc.tile_pool(name="zsbp", bufs=3))
        zNpool = es.enter_context(tc.tile_pool(name="zNp", bufs=3))
        aNpool = es.enter_context(tc.tile_pool(name="aNp", bufs=3))
        statpool = es.enter_context(tc.tile_pool(name="statp", bufs=4))
        rspool = es.enter_context(tc.tile_pool(name="rsp", bufs=4))
        tailpool = es.enter_context(tc.tile_pool(name="tailp", bufs=2))
        zpsum = es.enter_context(tc.tile_pool(name="zps", bufs=2, space="PSUM"))
        aggpsum = es.enter_context(
            tc.tile_pool(name="aggps", bufs=2, space="PSUM"))
        cspsum = es.enter_context(tc.tile_pool(name="csps", bufs=1, space="PSUM"))
        tailpsum = es.enter_context(tc.tile_pool(name="tps", bufs=1, space="PSUM"))

        Wbf = singles.tile([K, D], BF16)
        nc.gpsimd.dma_start(out=Wbf, in_=W[:, :])
        # WTs[p, c, k] = W[k, 128c + p]  (W^T d-chunks, PE stationaries)
        WTs = singles.tile([128, 8, K], BF16)
        nc.sync.dma_start(out=WTs, in_=Wbf, transpose=True)
        cpair = singles.tile([128, D], F32)
        nc.gpsimd.dma_start(out=cpair[0:64, :], in_=C[:, :])
        nc.gpsimd.dma_start(out=cpair[64:128, :], in_=C[:, :])
        ind2 = singles.tile([2, 128], F32)
        nc.sync.dma_start(out=ind2, in_=ind2_d[:, :])
        indK = singles.tile([128, 2], F32)
        nc.sync.dma_start(out=indK, in_=indK_d[:, :])

        def quarter(s, q, agg, cs):
            base = 64 * (s % 2)
            first = q == 0
            last = q == NQ - 1
            # ---- load quarter: xq[p, t, d] = x[s, q*512 + 4p + t, d]
            xq = xqpool.tile([128, TQ, D], BF16, tag="xq", name=f"xq_{s}_{q}")
            nc.gpsimd.dma_start(out=xq, in_=xr[s, q])
            # ---- ssq[p, t] = sum_d x^2 (split ACT / DVE by quarter parity)
            ssq = statpool.tile([128, TQ], F32, tag="ssq", name=f"ssq_{s}_{q}")
            for t in range(TQ):
                sq = sqpool.tile([128, D], BF16, tag="sq", name=f"sq_{s}_{q}_{t}")
                if (TQ * q + t) % 8 < 4:
                    nc.vector.tensor_mul(sq, xq[:, t, :], xq[:, t, :])
                    nc.vector.tensor_scalar(
                        out=sq, in0=sq, scalar1=1.0, scalar2=0.0,
                        op0=OP.mult, op1=OP.add, accum_out=ssq[:, t:t + 1],
                    )
                else:
                    nc.scalar.activation(
                        out=sq, in_=xq[:, t, :], func=AF.Square,
                        accum_out=ssq[:, t:t + 1],
                    )
            # ---- r = 1/||x||, invr = ||x|| (exp/ln chain)
            lnt = statpool.tile([128, TQ], F32, tag="lnt", name=f"lnt_{s}_{q}")
            nc.scalar.activation(out=lnt, in_=ssq, func=AF.Ln)
            r = statpool.tile([128, TQ], F32, tag="r", name=f"r_{s}_{q}")
            nc.scalar.activation(out=r, in_=lnt, func=AF.Exp, scale=-0.5)
            invr = statpool.tile([128, TQ], BF16, tag="invr",
                                 name=f"invr_{s}_{q}")
            nc.scalar.activation(out=invr, in_=lnt, func=AF.Exp, scale=0.5)
            # ---- transpose quarter: xT[p, t*8+dc, j] = x[m=q*512+4j+t, dc*128+p]
            xT = xTpool.tile([128, 8 * TQ, 128], BF16, tag="xT",
                             name=f"xT_{s}_{q}")
            nc.sync.dma_start(out=xT, in_=xq[:, :, :], transpose=True)
            # ---- zT[k, t*128 + j] = sum_d W[k, d] x[m(j, t), d]
            zq = zpsum.tile([K, TQ * 128], F32, tag="zq", name=f"zq_{s}_{q}")
            for dc in range(8):
                nc.tensor.matmul(
                    zq[:, :], lhsT=WTs[:, dc, :], rhs=xT[:, dc::8, :],
                    start=(dc == 0), stop=(dc == 7),
                )
            zsb = zsbpool.tile([K, TQ * 128], BF16, tag="zsb",
                               name=f"zsb_{s}_{q}")
            nc.vector.tensor_copy(out=zsb, in_=zq)
            # ---- back to natural layout: zN[p, t, k] = z[m=q*512+4p+t, k]
            zN = zNpool.tile([128, TQ, K], BF16, tag="zN", name=f"zN_{s}_{q}")
            nc.sync.dma_start(out=zN, in_=zsb, transpose=True)
            # ---- softmax: e = exp(r*z) with accum -> sden; a' = e * r/sden
            sden = statpool.tile([128, TQ], F32, tag="sden", name=f"sden_{s}_{q}")
            eN = aNpool.tile([128, TQ, K], BF16, tag="eN", name=f"eN_{s}_{q}")
            for t in range(TQ):
                nc.scalar.activation(
                    out=eN[:, t, :], in_=zN[:, t, :], func=AF.Exp,
                    scale=r[:, t:t + 1], accum_out=sden[:, t:t + 1],
                )
            lns = statpool.tile([128, TQ], F32, tag="lns", name=f"lns_{s}_{q}")
            nc.scalar.activation(out=lns, in_=sden, func=AF.Ln)
            invS = statpool.tile([128, TQ], F32, tag="invS",
                                 name=f"invS_{s}_{q}")
            nc.scalar.activation(out=invS, in_=lns, func=AF.Exp, scale=-1.0)
            rs = statpool.tile([128, TQ], F32, tag="rs", name=f"rs_{s}_{q}")
            nc.vector.tensor_mul(rs, r, invS)
            aN = aNpool.tile([128, TQ, K], BF16, tag="aN", name=f"aN_{s}_{q}")
            for t in range(TQ):
                nc.vector.tensor_scalar_mul(
                    aN[:, t, :], eN[:, t, :], rs[:, t:t + 1])
            # ---- aggregate: agg += a'^T @ x_raw ; cs += a'^T @ ||x||
            for t in range(TQ):
                st_ = first and t == 0
                sp_ = last and t == TQ - 1
                nc.tensor.matmul(
                    agg[base:base + 64, 0:512], lhsT=aN[:, t, :],
                    rhs=xq[:, t, 0:512], start=st_, stop=sp_,
                )
                nc.tensor.matmul(
                    agg[base:base + 64, 512:1024], lhsT=aN[:, t, :],
                    rhs=xq[:, t, 512:1024], start=st_, stop=sp_,
                )
                nc.tensor.matmul(
                    cs[base:base + 64, 0:1], lhsT=aN[:, t, :],
                    rhs=invr[:, t:t + 1], start=st_, stop=sp_,
                )

        def tail_pair(p, agg, cs):
            sa, sb = 2 * p, 2 * p + 1
            cssb = rspool.tile([128, 1], F32, tag="cssb", name=f"cssb_{p}")
            nc.vector.tensor_copy(out=cssb, in_=cs[:, 0:1])
            tmp = tailpool.tile([128, D], F32, tag="tmp", name=f"tmp_{p}")
            nc.vector.tensor_scalar_mul(tmp, cpair, cssb)
            vlad = tailpool.tile([128, D], F32, tag="vlad", name=f"vlad_{p}")
            nc.vector.tensor_sub(vlad, agg[:, :], tmp)
            sq2 = sqpool.tile([128, D], BF16, tag="sq", name=f"sqt_{p}")
            vssq = rspool.tile([128, 1], F32, tag="vssq", name=f"vssq_{p}")
            nc.scalar.activation(out=sq2, in_=vlad, func=AF.Square,
                                 accum_out=vssq)
            lnv = rspool.tile([128, 1], F32, tag="lnv", name=f"lnv_{p}")
            nc.scalar.activation(out=lnv, in_=vssq, func=AF.Ln)
            rv = rspool.tile([128, 1], F32, tag="rv", name=f"rv_{p}")
            nc.scalar.activation(out=rv, in_=lnv, func=AF.Exp, scale=-0.5)
            ssqn = rspool.tile([128, 1], F32, tag="ssqn", name=f"ssqn_{p}")
            nc.vector.tensor_scalar(
                out=ssqn, in0=vssq, scalar1=rv, scalar2=rv,
                op0=OP.mult, op1=OP.mult,
            )
            gsum = tailpsum.tile([2, 2], F32, tag="tps", name=f"gsum_{p}")
            nc.tensor.matmul(gsum[:, 0:1], lhsT=indK, rhs=ssqn,
                             start=True, stop=True)
            lng = rspool.tile([2, 1], F32, tag="lng", name=f"lng_{p}")
            nc.scalar.activation(out=lng, in_=gsum[:, 0:1], func=AF.Ln)
            ginv = rspool.tile([2, 1], F32, tag="ginv", name=f"ginv_{p}")
            nc.scalar.activation(out=ginv, in_=lng, func=AF.Exp, scale=-0.5)
            gb = tailpsum.tile([128, 2], F32, tag="tps", name=f"gb_{p}")
            nc.tensor.matmul(gb[:, 0:1], lhsT=ind2, rhs=ginv,
                             start=True, stop=True)
            fs = rspool.tile([128, 1], F32, tag="fs", name=f"fs_{p}")
            nc.vector.tensor_mul(fs, rv, gb[:, 0:1])
            osb = tailpool.tile([128, D], F32, tag="osb", name=f"osb_{p}")
            nc.vector.tensor_scalar_mul(osb, vlad, fs)
            nc.scalar.dma_start(out=outr[sa], in_=osb[0:64, :])
            nc.scalar.dma_start(out=outr[sb], in_=osb[64:128, :])

        for p in range(2):
            agg = aggpsum.tile([128, D], F32, tag="agg", name=f"agg_{p}")
            cs = cspsum.tile([128, 8], F32, tag="cs", name=f"cs_{p}")
            for s in (2 * p, 2 * p + 1):
                for q in range(NQ):
                    quarter(s, q, agg, cs)
            tail_pair(p, agg, cs)

    return nc


_NC_CACHE = None


def kernel(**inputs: np.ndarray) -> np.ndarray:
    global _NC_CACHE
    _apply_patch()
    from concourse.bass_utils import run_bass_kernel_spmd

    x = np.ascontiguousarray(np.asarray(inputs["x"], dtype=np.float32))
    W = np.ascontiguousarray(np.asarray(inputs["W"], dtype=np.float32))
    cent = np.ascontiguousarray(
        np.asarray(inputs["centroids"], dtype=np.float32))

    ind2 = np.zeros((2, 128), dtype=np.float32)
    ind2[0, 0:64] = 1.0
    ind2[1, 64:128] = 1.0
    indK = np.zeros((128, 2), dtype=np.float32)
    indK[0:64, 0] = 1.0
    indK[64:128, 1] = 1.0

    if _NC_CACHE is None:
        _NC_CACHE = build_nc()
    nc = _NC_CACHE

    in_maps = [
        {
            "x": np.ascontiguousarray(
                x[B_PER_CORE * c:B_PER_CORE * (c + 1)]),
            "W": W,
            "centroids": cent,
            "ind2": ind2,
            "indK": indK,
        }
        for c in range(N_CORES)
    ]
    res = run_bass_kernel_spmd(nc, in_maps, core_ids=list(range(N_CORES)))
    return np.concatenate([r["out"] for r in res.results], axis=0)


# revision 7
# speedup vs baseline: 1.1278x; 1.1215x over previous
"""NetVLAD on 8 Trainium2 NeuronCores — self-contained kernel.

Problem: x [32, 2048, 1024] f32, W [64, 1024] f32, centroids [64, 1024] f32
  -> out [32, 65536] f32  (NetVLAD pooling: per-frame L2 norm, soft-assign
  softmax over 64 clusters, residual aggregation, intra + global L2 norm).

Sharding: data-parallel over batch — 4 samples per core, W/centroids
replicated; no cross-core communication. Per-core program:
  x is DMA-cast to fp8e4 on load; the logits matmul needs x^T, produced by
  xbar DMA transposes of x viewed as fp16 *pairs* (halves xbar payload);
  each partition of x^T then holds two interleaved d-values, consumed by
  paired strided plain-fp8 matmuls against host-prepacked W columns.
  zT [64,512] psum -> bf16 sbuf -> PE identity-transpose back to natural
  [m,64] psum (keeps the xbar hazard chain free of data-dependent links),
  then a' = exp(r*z)*512*r/sum_k exp(r*z) on ACT/DVE, and
  agg += a'^T @ x_fp8, cs += a'^T @ ||x||  (PE, col-packed sample pairs);
  vlad = agg - cs*c; per-row + global L2 norm (the 512 scale cancels).
"""

import json

import numpy as np

import concourse.bass as bass
import concourse.mybir as mybir
import concourse.tile as tile

F32 = mybir.dt.float32
BF16 = mybir.dt.bfloat16
FP16 = mybir.dt.float16
FP8 = mybir.dt.float8e4
AF = mybir.ActivationFunctionType
OP = mybir.AluOpType

B = 32
N_CORES = 8
B_PER_CORE = B // N_CORES
M = 2048
D = 1024
K = 64
NQ = 4           # quarters per sample (512 m each)
TQ = 4           # m sub-tiles per quarter (128 m each)
LN512 = float(np.log(512.0))   # fp8 underflow guard on a'; cancels in norms

_PATCHED = False


def _split_waits_json(bir: dict, max_waits: int = 1) -> dict:
    """Split multi-wait sync infos into standalone EventSemaphore waits.

    The walrus build in this image supports a single sync-wait command per
    instruction, while Tile's sem assignment emits several (e.g. the
    kernel-tail Drain waits on every DMAHW lane). Hoisting the extra waits
    into preceding single-wait EventSemaphore instructions on the same
    engine is semantics-preserving for monotonic semaphores.
    """
    ctr = 0
    for f in bir.get("functions", []):
        for blk in f.get("blocks", []):
            insts = blk.get("instructions", [])
            new = []
            for inst in insts:
                si = inst.get("sync_info")
                waits = si.get("on_wait", []) if si else []
                if len(waits) > max_waits:
                    head, keep = waits[:-max_waits], waits[-max_waits:]
                    for w in head:
                        ctr += 1
                        new.append({
                            "debug": inst.get("debug", 0),
                            "engine": inst["engine"],
                            "ins": [],
                            "name": f"{inst['name']}-wsplit{ctr}",
                            "opcode": "EventSemaphore",
                            "outs": [],
                            "sync_info": {"on_update": [], "on_wait": [w]},
                        })
                    si["on_wait"] = keep
                new.append(inst)
            blk["instructions"] = new
    return bir


def _apply_patch():
    global _PATCHED
    if _PATCHED:
        return
    import concourse.bass_utils as bu
    import concourse.bass2jax as b2j
    orig = bu.compile_bir_kernel

    def patched(bir_json, tmpdir, neff_name="file.neff"):
        d = json.loads(bir_json)
        d = _split_waits_json(d, 1)
        return orig(json.dumps(d).encode(), tmpdir, neff_name)

    bu.compile_bir_kernel = patched
    b2j.compile_bir_kernel = patched
    _PATCHED = True


def build_nc():
    nc = bass.Bass()
    x = nc.dram_tensor("x", [B_PER_CORE, M, D], F32, kind="ExternalInput")
    C = nc.dram_tensor("centroids", [K, D], F32, kind="ExternalInput")
    out = nc.dram_tensor("out", [B_PER_CORE, K * D], F32, kind="ExternalOutput")
    # host-prepacked: Wpk[p, hc, j2, k] = W[k, 256*hc + 2*p + j2] in fp8
    Wpk_d = nc.dram_tensor("Wpk", [128, 4, 2, K], FP8, kind="ExternalInput")
    ident_d = nc.dram_tensor("ident", [K, K], F32, kind="ExternalInput")
    ind2_d = nc.dram_tensor("ind2", [2, 128], F32, kind="ExternalInput")
    indK_d = nc.dram_tensor("indK", [128, 2], F32, kind="ExternalInput")

    # m = q*512 + p*4 + t  -> 16KB contiguous HBM read per partition
    xr = x[:, :, :].rearrange("s (q p t) d -> s q p t d", q=NQ, p=128, t=TQ)
    outr = out[:, :].rearrange("s (k d) -> s k d", d=D)

    from contextlib import ExitStack
    with tile.TileContext(nc) as tc, ExitStack() as es:
        singles = es.enter_context(tc.tile_pool(name="singles", bufs=1))
        xqpool = es.enter_context(tc.tile_pool(name="xqp", bufs=6))
        xTpool = es.enter_context(tc.tile_pool(name="xTp", bufs=4))
        sqpool = es.enter_context(tc.tile_pool(name="sqp", bufs=2))
        zsbpool = es.enter_context(tc.tile_pool(name="zsbp", bufs=3))
        aNpool = es.enter_context(tc.tile_pool(name="aNp", bufs=3))
        statpool = es.enter_context(tc.tile_pool(name="statp", bufs=4))
        rspool = es.enter_context(tc.tile_pool(name="rsp", bufs=4))
        tailpool = es.enter_context(tc.tile_pool(name="tailp", bufs=2))
        zpsum = es.enter_context(tc.tile_pool(name="zps", bufs=1, space="PSUM"))
        zNpsum = es.enter_context(tc.tile_pool(name="zNps", bufs=1,
                                               space="PSUM"))
        aggpsum = es.enter_context(
            tc.tile_pool(name="aggps", bufs=2, space="PSUM"))
        cspsum = es.enter_context(tc.tile_pool(name="csps", bufs=1, space="PSUM"))

        Wpk = singles.tile([128, 4, 2, K], FP8)
        nc.gpsimd.dma_start(out=Wpk, in_=Wpk_d[:, :, :, :])
        ident = singles.tile([K, K], BF16)
        nc.gpsimd.dma_start(out=ident, in_=ident_d[:, :])
        cpair = singles.tile([128, D], F32)
        nc.gpsimd.dma_start(out=cpair[0:64, :], in_=C[:, :])
        nc.gpsimd.dma_start(out=cpair[64:128, :], in_=C[:, :])
        ind2 = singles.tile([2, 128], F32)
        nc.gpsimd.dma_start(out=ind2, in_=ind2_d[:, :])
        indK = singles.tile([128, 2], F32)
        nc.gpsimd.dma_start(out=indK, in_=indK_d[:, :])

        def quarter(s, q, agg, cs):
            base = 64 * (s % 2)
            first = q == 0
            last = q == NQ - 1
            # ---- load quarter: xq[p, t, d] = fp8(x[s, q*512 + 4p + t, d])
            xq = xqpool.tile([128, TQ, D], FP8, tag="xq", name=f"xq_{s}_{q}")
            nc.gpsimd.dma_start(out=xq, in_=xr[s, q])
            # ---- ssq[p, t] = sum_d x^2 (split ACT / DVE by quarter parity)
            ssq = statpool.tile([128, TQ], F32, tag="ssq", name=f"ssq_{s}_{q}")
            for t in range(TQ):
                sq = sqpool.tile([128, D], BF16, tag="sq", name=f"sq_{s}_{q}_{t}")
                if (TQ * q + t) % 8 < 4:
                    nc.vector.tensor_mul(sq, xq[:, t, :], xq[:, t, :])
                    nc.vector.tensor_scalar(
                        out=sq, in0=sq, scalar1=1.0, scalar2=0.0,
                        op0=OP.mult, op1=OP.add, accum_out=ssq[:, t:t + 1],
                    )
                else:
                    nc.scalar.activation(
                        out=sq, in_=xq[:, t, :], func=AF.Square,
                        accum_out=ssq[:, t:t + 1],
                    )
            # ---- r = 1/||x||, invr = ||x|| (exp/ln chain)
            lnt = statpool.tile([128, TQ], F32, tag="lnt", name=f"lnt_{s}_{q}")
            nc.scalar.activation(out=lnt, in_=ssq, func=AF.Ln)
            r = statpool.tile([128, TQ], F32, tag="r", name=f"r_{s}_{q}")
            nc.scalar.activation(out=r, in_=lnt, func=AF.Exp, scale=-0.5)
            invr = statpool.tile([128, TQ], FP8, tag="invr",
                                 name=f"invr_{s}_{q}")
            nc.scalar.activation(out=invr, in_=lnt, func=AF.Exp, scale=0.5)
            # ---- transpose quarter as fp16 pairs:
            # xT16[p, t*4+hc, j] = pair x[m(j,t), d = 256*hc + 2p + {0,1}]
            xT16 = xTpool.tile([128, 4 * TQ, 128], FP16, tag="xT",
                               name=f"xT_{s}_{q}")
            nc.sync.dma_start(out=xT16, in_=xq[:, :, :].bitcast(FP16),
                              transpose=True)
            xT8 = xT16[:, :, :].bitcast(FP8)   # [128, 16, 256]
            # ---- zT[k, t*128 + j] = sum_d W[k, d] x[m(j, t), d]
            zq = zpsum.tile([K, TQ * 128], F32, tag="zq", name=f"zq_{s}_{q}")
            mm = 0
            for hc in range(4):
                for j2 in range(2):
                    nc.tensor.matmul(
                        zq[:, :], lhsT=Wpk[:, hc, j2, :],
                        rhs=xT8[:, hc::4, j2::2],
                        start=(mm == 0), stop=(mm == 7),
                    )
                    mm += 1
            zsb = zsbpool.tile([K, TQ * 128], BF16, tag="zsb",
                               name=f"zsb_{s}_{q}")
            nc.vector.tensor_copy(out=zsb, in_=zq)
            # ---- back to natural layout via PE transpose (stays off the
            # xbar hazard chain): zN[p, t, k] = z[m=q*512+4p+t, k]
            zN = zNpsum.tile([128, TQ, K], BF16, tag="zN", name=f"zN_{s}_{q}")
            for t in range(TQ):
                nc.tensor.transpose(
                    zN[:, t, :], zsb[:, 128 * t:128 * (t + 1)], ident)
            # ---- softmax: e = exp(r*z) with accum -> sden; a' = e * 512*r/sden
            sden = statpool.tile([128, TQ], F32, tag="sden", name=f"sden_{s}_{q}")
            eN = aNpool.tile([128, TQ, K], BF16, tag="eN", name=f"eN_{s}_{q}")
            for t in range(TQ):
                nc.scalar.activation(
                    out=eN[:, t, :], in_=zN[:, t, :], func=AF.Exp,
                    scale=r[:, t:t + 1], accum_out=sden[:, t:t + 1],
                )
            lns = statpool.tile([128, TQ], F32, tag="lns", name=f"lns_{s}_{q}")
            nc.scalar.activation(out=lns, in_=sden, func=AF.Ln)
            invS = statpool.tile([128, TQ], F32, tag="invS",
                                 name=f"invS_{s}_{q}")
            nc.scalar.activation(out=invS, in_=lns, func=AF.Exp, scale=-1.0)
            rs = statpool.tile([128, TQ], F32, tag="rs", name=f"rs_{s}_{q}")
            nc.vector.tensor_mul(rs, r, invS)
            aN = aNpool.tile([128, TQ, K], FP8, tag="aN", name=f"aN_{s}_{q}")
            for t in range(TQ):
                nc.vector.tensor_scalar(
                    out=aN[:, t, :], in0=eN[:, t, :],
                    scalar1=rs[:, t:t + 1], scalar2=512.0,
                    op0=OP.mult, op1=OP.mult,
                )
            # ---- aggregate: agg += a'^T @ x_fp8 ; cs += a'^T @ ||x||
            for t in range(TQ):
                st_ = first and t == 0
                sp_ = last and t == TQ - 1
                nc.tensor.matmul(
                    agg[base:base + 64, 0:512], lhsT=aN[:, t, :],
                    rhs=xq[:, t, 0:512], start=st_, stop=sp_,
                )
                nc.tensor.matmul(
                    agg[base:base + 64, 512:1024], lhsT=aN[:, t, :],
                    rhs=xq[:, t, 512:1024], start=st_, stop=sp_,
                )
                nc.tensor.matmul(
                    cs[base:base + 64, 0:1], lhsT=aN[:, t, :],
                    rhs=invr[:, t:t + 1], start=st_, stop=sp_,
                )

        def tail_pair(p, agg, cs):
            sa, sb = 2 * p, 2 * p + 1
            cssb = rspool.tile([128, 1], F32, tag="cssb", name=f"cssb_{p}")
            nc.vector.tensor_copy(out=cssb, in_=cs[:, 0:1])
            tmp = tailpool.tile([128, D], F32, tag="tmp", name=f"tmp_{p}")
            nc.vector.tensor_scalar_mul(tmp, cpair, cssb)
            vlad = tailpool.tile([128, D], F32, tag="vlad", name=f"vlad_{p}")
            nc.vector.tensor_sub(vlad, agg[:, :], tmp)
            sq2 = sqpool.tile([128, D], BF16, tag="sq", name=f"sqt_{p}")
            vssq = rspool.tile([128, 1], F32, tag="vssq", name=f"vssq_{p}")
            nc.scalar.activation(out=sq2, in_=vlad, func=AF.Square,
                                 accum_out=vssq)
            lnv = rspool.tile([128, 1], F32, tag="lnv", name=f"lnv_{p}")
            nc.scalar.activation(out=lnv, in_=vssq, func=AF.Ln)
            rv = rspool.tile([128, 1], F32, tag="rv", name=f"rv_{p}")
            nc.scalar.activation(out=rv, in_=lnv, func=AF.Exp, scale=-0.5)
            ssqn = rspool.tile([128, 1], F32, tag="ssqn", name=f"ssqn_{p}")
            nc.vector.tensor_scalar(
                out=ssqn, in0=vssq, scalar1=rv, scalar2=rv,
                op0=OP.mult, op1=OP.mult,
            )
            gsum = cspsum.tile([2, 2], F32, tag="tps", name=f"gsum_{p}")
            nc.tensor.matmul(gsum[:, 0:1], lhsT=indK, rhs=ssqn,
                             start=True, stop=True)
            lng = rspool.tile([2, 1], F32, tag="lng", name=f"lng_{p}")
            nc.scalar.activation(out=lng, in_=gsum[:, 0:1], func=AF.Ln)
            ginv = rspool.tile([2, 1], F32, tag="ginv", name=f"ginv_{p}")
            nc.scalar.activation(out=ginv, in_=lng, func=AF.Exp, scale=-0.5)
            gb = cspsum.tile([128, 2], F32, tag="tps", name=f"gb_{p}")
            nc.tensor.matmul(gb[:, 0:1], lhsT=ind2, rhs=ginv,
                             start=True, stop=True)
            fs = rspool.tile([128, 1], F32, tag="fs", name=f"fs_{p}")
            nc.vector.tensor_mul(fs, rv, gb[:, 0:1])
            osb = tailpool.tile([128, D], F32, tag="osb", name=f"osb_{p}")
            nc.vector.tensor_scalar_mul(osb, vlad, fs)
            nc.scalar.dma_start(out=outr[sa], in_=osb[0:64, :])
            nc.scalar.dma_start(out=outr[sb], in_=osb[64:128, :])

        for p in range(2):
            agg = aggpsum.tile([128, D], F32, tag="agg", name=f"agg_{p}")
            cs = cspsum.tile([128, 8], F32, tag="cs", name=f"cs_{p}")
            for s in (2 * p, 2 * p + 1):
                for q in range(NQ):
                    quarter(s, q, agg, cs)
            tail_pair(p, agg, cs)

    return nc


_NC_CACHE = None


def make_in_maps(inputs: dict) -> list:
    x = np.ascontiguousarray(np.asarray(inputs["x"], dtype=np.float32))
    W = np.ascontiguousarray(np.asarray(inputs["W"], dtype=np.float32))
    cent = np.ascontiguousarray(
        np.asarray(inputs["centroids"], dtype=np.float32))

    np8 = mybir.dt.np(FP8)
    # Wpk[p, hc, j2, k] = W[k, 256*hc + 2*p + j2]
    Wr = W.reshape(K, 4, 128, 2)                  # [k, hc, p, j2]
    Wpk = np.ascontiguousarray(
        Wr.transpose(2, 1, 3, 0)).astype(np8)     # [p, hc, j2, k]
    ident = np.eye(K, dtype=np.float32)

    ind2 = np.zeros((2, 128), dtype=np.float32)
    ind2[0, 0:64] = 1.0
    ind2[1, 64:128] = 1.0
    indK = np.zeros((128, 2), dtype=np.float32)
    indK[0:64, 0] = 1.0
    indK[64:128, 1] = 1.0

    return [
        {
            "x": np.ascontiguousarray(
                x[B_PER_CORE * c:B_PER_CORE * (c + 1)]),
            "Wpk": Wpk,
            "centroids": cent,
            "ident": ident,
            "ind2": ind2,
            "indK": indK,
        }
        for c in range(N_CORES)
    ]


def kernel(**inputs: np.ndarray) -> np.ndarray:
    global _NC_CACHE
    _apply_patch()
    from concourse.bass_utils import run_bass_kernel_spmd

    if _NC_CACHE is None:
        _NC_CACHE = build_nc()
    nc = _NC_CACHE

    in_maps = make_in_maps(inputs)
    res = run_bass_kernel_spmd(nc, in_maps, core_ids=list(range(N_CORES)))
    return np.concatenate([r["out"] for r in res.results], axis=0)


# revision 9
# speedup vs baseline: 1.2746x; 1.1302x over previous
"""NetVLAD on 8 Trainium2 NeuronCores — self-contained kernel.

Problem: x [32, 2048, 1024] f32, W [64, 1024] f32, centroids [64, 1024] f32
  -> out [32, 65536] f32  (NetVLAD pooling: per-frame L2 norm, soft-assign
  softmax over 64 clusters, residual aggregation, intra + global L2 norm).

Sharding: data-parallel over batch — 4 samples per core, W/centroids
replicated; no cross-core communication. Per-core pipeline (per 512-row
quarter): bf16 DMA-cast load -> squares (ACT/GPSIMD/DVE split) -> one-op
DVE cast to fp8 -> xbar DMA transpose of the fp8 copy viewed as fp16
*pairs* (halves xbar payload; ~190 GB/s effective) -> paired strided
plain-fp8 matmuls vs host-prepacked W columns -> zT [64,512] -> bf16 ->
PE identity-transpose to natural [m,64] (keeps the xbar hazard chain free
of data-dependent links) -> exp(r*z) with accум -> a' = e*r/S -> agg/cs
matmuls (deferred one quarter so they don't block the next quarter's z
matmuls in the PE FIFO). vlad = agg - cs*c; per-row + global L2 norm.
"""

import json

import numpy as np

import concourse.bass as bass
import concourse.mybir as mybir
import concourse.tile as tile

F32 = mybir.dt.float32
BF16 = mybir.dt.bfloat16
FP16 = mybir.dt.float16
FP8 = mybir.dt.float8e4
AF = mybir.ActivationFunctionType
OP = mybir.AluOpType

B = 32
N_CORES = 8
B_PER_CORE = B // N_CORES
M = 2048
D = 1024
K = 64
NQ = 4           # quarters per sample (512 m each)
TQ = 4           # m sub-tiles per quarter (128 m each)

_PATCHED = False


def _split_waits_json(bir: dict, max_waits: int = 1) -> dict:
    """Split multi-wait sync infos into standalone EventSemaphore waits.

    The walrus build in this image supports a single sync-wait command per
    instruction, while Tile's sem assignment emits several (e.g. the
    kernel-tail Drain waits on every DMAHW lane). Hoisting the extra waits
    into preceding single-wait EventSemaphore instructions on the same
    engine is semantics-preserving for monotonic semaphores.
    """
    ctr = 0
    for f in bir.get("functions", []):
        for blk in f.get("blocks", []):
            insts = blk.get("instructions", [])
            new = []
            for inst in insts:
                si = inst.get("sync_info")
                waits = si.get("on_wait", []) if si else []
                if len(waits) > max_waits:
                    head, keep = waits[:-max_waits], waits[-max_waits:]
                    for w in head:
                        ctr += 1
                        new.append({
                            "debug": inst.get("debug", 0),
                            "engine": inst["engine"],
                            "ins": [],
                            "name": f"{inst['name']}-wsplit{ctr}",
                            "opcode": "EventSemaphore",
                            "outs": [],
                            "sync_info": {"on_update": [], "on_wait": [w]},
                        })
                    si["on_wait"] = keep
                new.append(inst)
            blk["instructions"] = new
    return bir


def _apply_patch():
    global _PATCHED
    if _PATCHED:
        return
    import concourse.bass_utils as bu
    import concourse.bass2jax as b2j
    orig = bu.compile_bir_kernel

    def patched(bir_json, tmpdir, neff_name="file.neff"):
        d = json.loads(bir_json)
        d = _split_waits_json(d, 1)
        return orig(json.dumps(d).encode(), tmpdir, neff_name)

    bu.compile_bir_kernel = patched
    b2j.compile_bir_kernel = patched
    _PATCHED = True


def build_nc():
    nc = bass.Bass()
    x = nc.dram_tensor("x", [B_PER_CORE, M, D], F32, kind="ExternalInput")
    C = nc.dram_tensor("centroids", [K, D], F32, kind="ExternalInput")
    out = nc.dram_tensor("out", [B_PER_CORE, K * D], F32, kind="ExternalOutput")
    # host-prepacked: Wpk[p, hc, j2, k] = W[k, 256*hc + 2*p + j2] in fp8
    Wpk_d = nc.dram_tensor("Wpk", [128, 4, 2, K], FP8, kind="ExternalInput")
    ident_d = nc.dram_tensor("ident", [K, K], F32, kind="ExternalInput")
    ind2_d = nc.dram_tensor("ind2", [2, 128], F32, kind="ExternalInput")
    indK_d = nc.dram_tensor("indK", [128, 2], F32, kind="ExternalInput")

    # m = q*512 + p*4 + t  -> 16KB contiguous HBM read per partition
    xr = x[:, :, :].rearrange("s (q p t) d -> s q p t d", q=NQ, p=128, t=TQ)
    outr = out[:, :].rearrange("s (k d) -> s k d", d=D)

    from contextlib import ExitStack
    with tile.TileContext(nc) as tc, ExitStack() as es:
        singles = es.enter_context(tc.tile_pool(name="singles", bufs=1))
        xqpool = es.enter_context(tc.tile_pool(name="xqp", bufs=5))
        xq8pool = es.enter_context(tc.tile_pool(name="xq8p", bufs=4))
        xTpool = es.enter_context(tc.tile_pool(name="xTp", bufs=4))
        sqpool = es.enter_context(tc.tile_pool(name="sqp", bufs=3))
        zsbpool = es.enter_context(tc.tile_pool(name="zsbp", bufs=3))
        aNpool = es.enter_context(tc.tile_pool(name="aNp", bufs=4))
        statpool = es.enter_context(tc.tile_pool(name="statp", bufs=4))
        rspool = es.enter_context(tc.tile_pool(name="rsp", bufs=4))
        tailpool = es.enter_context(tc.tile_pool(name="tailp", bufs=2))
        zpsum = es.enter_context(tc.tile_pool(name="zps", bufs=1, space="PSUM"))
        zNpsum = es.enter_context(tc.tile_pool(name="zNps", bufs=1,
                                               space="PSUM"))
        aggpsum = es.enter_context(
            tc.tile_pool(name="aggps", bufs=2, space="PSUM"))
        cspsum = es.enter_context(tc.tile_pool(name="csps", bufs=1, space="PSUM"))

        Wpk = singles.tile([128, 4, 2, K], FP8)
        nc.gpsimd.dma_start(out=Wpk, in_=Wpk_d[:, :, :, :])
        ident = singles.tile([K, K], BF16)
        nc.gpsimd.dma_start(out=ident, in_=ident_d[:, :])
        cpair = singles.tile([128, D], F32)
        nc.gpsimd.dma_start(out=cpair[0:64, :], in_=C[:, :])
        nc.gpsimd.dma_start(out=cpair[64:128, :], in_=C[:, :])
        ind2 = singles.tile([2, 128], F32)
        nc.gpsimd.dma_start(out=ind2, in_=ind2_d[:, :])
        indK = singles.tile([128, 2], F32)
        nc.gpsimd.dma_start(out=indK, in_=indK_d[:, :])

        def quarter(s, q):
            """Produce stage: everything except the agg/cs matmuls."""
            # ---- load quarter: xq[p, t, d] = bf16(x[s, q*512 + 4p + t, d])
            xq = xqpool.tile([128, TQ, D], BF16, tag="xq", name=f"xq_{s}_{q}")
            nc.gpsimd.dma_start(out=xq, in_=xr[s, q])
            # ---- one-op fp8 copy (feeds transpose + z-matmul only)
            xq8 = xq8pool.tile([128, TQ, D], FP8, tag="xq8", name=f"xq8_{s}_{q}")
            nc.vector.tensor_copy(out=xq8, in_=xq)
            # ---- ssq[p, t] = sum_d x^2 (ACT / GPSIMD / DVE split)
            ssq = statpool.tile([128, TQ], F32, tag="ssq", name=f"ssq_{s}_{q}")
            for t in range(TQ):
                sq = sqpool.tile([128, D], BF16, tag="sq", name=f"sq_{s}_{q}_{t}")
                r4 = (TQ * q + t) % 4
                if r4 != 3:
                    nc.scalar.activation(
                        out=sq, in_=xq[:, t, :], func=AF.Square,
                        accum_out=ssq[:, t:t + 1],
                    )
                else:
                    nc.vector.tensor_mul(sq, xq[:, t, :], xq[:, t, :])
                    nc.vector.tensor_scalar(
                        out=sq, in0=sq, scalar1=1.0, scalar2=0.0,
                        op0=OP.mult, op1=OP.add, accum_out=ssq[:, t:t + 1],
                    )
            # ---- r = 1/||x||, invr = ||x|| = ssq * r
            lnt = statpool.tile([128, TQ], F32, tag="lnt", name=f"lnt_{s}_{q}")
            nc.scalar.activation(out=lnt, in_=ssq, func=AF.Ln)
            r = statpool.tile([128, TQ], F32, tag="r", name=f"r_{s}_{q}")
            nc.scalar.activation(out=r, in_=lnt, func=AF.Exp, scale=-0.5)
            invr = statpool.tile([128, TQ], BF16, tag="invr",
                                 name=f"invr_{s}_{q}")
            nc.vector.tensor_mul(invr, ssq, r)
            # ---- transpose quarter as fp16 pairs:
            # xT16[p, t*4+hc, j] = pair x[m(j,t), d = 256*hc + 2p + {0,1}]
            xT16 = xTpool.tile([128, 4 * TQ, 128], FP16, tag="xT",
                               name=f"xT_{s}_{q}")
            nc.sync.dma_start(out=xT16, in_=xq8[:, :, :].bitcast(FP16),
                              transpose=True)
            xT8 = xT16[:, :, :].bitcast(FP8)   # [128, 16, 256]
            # ---- zT[k, t*128 + j] = sum_d W[k, d] x[m(j, t), d]
            zq = zpsum.tile([K, TQ * 128], F32, tag="zq", name=f"zq_{s}_{q}")
            mm = 0
            for hc in range(4):
                for j2 in range(2):
                    nc.tensor.matmul(
                        zq[:, :], lhsT=Wpk[:, hc, j2, :],
                        rhs=xT8[:, hc::4, j2::2],
                        start=(mm == 0), stop=(mm == 7),
                    )
                    mm += 1
            zsb = zsbpool.tile([K, TQ * 128], BF16, tag="zsb",
                               name=f"zsb_{s}_{q}")
            nc.vector.tensor_copy(out=zsb, in_=zq)
            # ---- back to natural layout via PE transpose (stays off the
            # xbar hazard chain): zN[p, t, k] = z[m=q*512+4p+t, k]
            zN = zNpsum.tile([128, TQ, K], BF16, tag="zN", name=f"zN_{s}_{q}")
            for t in range(TQ):
                nc.tensor.transpose(
                    zN[:, t, :], zsb[:, 128 * t:128 * (t + 1)], ident)
            # ---- softmax: e = exp(r*z) with accum -> sden; a' = e * r/sden
            sden = statpool.tile([128, TQ], F32, tag="sden", name=f"sden_{s}_{q}")
            eN = aNpool.tile([128, TQ, K], BF16, tag="eN", name=f"eN_{s}_{q}")
            for t in range(TQ):
                nc.scalar.activation(
                    out=eN[:, t, :], in_=zN[:, t, :], func=AF.Exp,
                    scale=r[:, t:t + 1], accum_out=sden[:, t:t + 1],
                )
            lns = statpool.tile([128, TQ], F32, tag="lns", name=f"lns_{s}_{q}")
            nc.scalar.activation(out=lns, in_=sden, func=AF.Ln)
            invS = statpool.tile([128, TQ], F32, tag="invS",
                                 name=f"invS_{s}_{q}")
            nc.scalar.activation(out=invS, in_=lns, func=AF.Exp, scale=-1.0)
            rs = statpool.tile([128, TQ], F32, tag="rs", name=f"rs_{s}_{q}")
            nc.vector.tensor_mul(rs, r, invS)
            aN = aNpool.tile([128, TQ, K], BF16, tag="aN", name=f"aN_{s}_{q}")
            for t in range(TQ):
                nc.vector.tensor_scalar_mul(
                    aN[:, t, :], eN[:, t, :], rs[:, t:t + 1])
            return (xq, invr, aN)

        def aggregate(work, agg, cs):
            """Consume stage: agg/cs matmuls for an earlier quarter."""
            (s, q, xq, invr, aN) = work
            base = 64 * (s % 2)
            first = q == 0
            last = q == NQ - 1
            for t in range(TQ):
                st_ = first and t == 0
                sp_ = last and t == TQ - 1
                nc.tensor.matmul(
                    agg[base:base + 64, 0:512], lhsT=aN[:, t, :],
                    rhs=xq[:, t, 0:512], start=st_, stop=sp_,
                )
                nc.tensor.matmul(
                    agg[base:base + 64, 512:1024], lhsT=aN[:, t, :],
                    rhs=xq[:, t, 512:1024], start=st_, stop=sp_,
                )
                nc.tensor.matmul(
                    cs[base:base + 64, 0:1], lhsT=aN[:, t, :],
                    rhs=invr[:, t:t + 1], start=st_, stop=sp_,
                )

        def tail_pair(p, agg, cs):
            sa, sb = 2 * p, 2 * p + 1
            cssb = rspool.tile([128, 1], F32, tag="cssb", name=f"cssb_{p}")
            nc.vector.tensor_copy(out=cssb, in_=cs[:, 0:1])
            tmp = tailpool.tile([128, D], F32, tag="tmp", name=f"tmp_{p}")
            nc.vector.tensor_scalar_mul(tmp, cpair, cssb)
            vlad = tailpool.tile([128, D], F32, tag="vlad", name=f"vlad_{p}")
            nc.vector.tensor_sub(vlad, agg[:, :], tmp)
            sq2 = sqpool.tile([128, D], BF16, tag="sq", name=f"sqt_{p}")
            vssq = rspool.tile([128, 1], F32, tag="vssq", name=f"vssq_{p}")
            nc.scalar.activation(out=sq2, in_=vlad, func=AF.Square,
                                 accum_out=vssq)
            lnv = rspool.tile([128, 1], F32, tag="lnv", name=f"lnv_{p}")
            nc.scalar.activation(out=lnv, in_=vssq, func=AF.Ln)
            rv = rspool.tile([128, 1], F32, tag="rv", name=f"rv_{p}")
            nc.scalar.activation(out=rv, in_=lnv, func=AF.Exp, scale=-0.5)
            ssqn = rspool.tile([128, 1], F32, tag="ssqn", name=f"ssqn_{p}")
            nc.vector.tensor_scalar(
                out=ssqn, in0=vssq, scalar1=rv, scalar2=rv,
                op0=OP.mult, op1=OP.mult,
            )
            gsum = cspsum.tile([2, 2], F32, tag="tps", name=f"gsum_{p}")
            nc.tensor.matmul(gsum[:, 0:1], lhsT=indK, rhs=ssqn,
                             start=True, stop=True)
            lng = rspool.tile([2, 1], F32, tag="lng", name=f"lng_{p}")
            nc.scalar.activation(out=lng, in_=gsum[:, 0:1], func=AF.Ln)
            ginv = rspool.tile([2, 1], F32, tag="ginv", name=f"ginv_{p}")
            nc.scalar.activation(out=ginv, in_=lng, func=AF.Exp, scale=-0.5)
            gb = cspsum.tile([128, 2], F32, tag="tps", name=f"gb_{p}")
            nc.tensor.matmul(gb[:, 0:1], lhsT=ind2, rhs=ginv,
                             start=True, stop=True)
            fs = rspool.tile([128, 1], F32, tag="fs", name=f"fs_{p}")
            nc.vector.tensor_mul(fs, rv, gb[:, 0:1])
            osb = tailpool.tile([128, D], F32, tag="osb", name=f"osb_{p}")
            nc.vector.tensor_scalar_mul(osb, vlad, fs)
            nc.scalar.dma_start(out=outr[sa], in_=osb[0:64, :])
            nc.scalar.dma_start(out=outr[sb], in_=osb[64:128, :])

        # software-pipelined: agg/cs of quarter i run after produce(i+1)
        pending = None
        pend_pair = {}
        for p in range(2):
            agg = aggpsum.tile([128, D], F32, tag="agg", name=f"agg_{p}")
            cs = cspsum.tile([128, 8], F32, tag="cs", name=f"cs_{p}")
            pend_pair[p] = (agg, cs)
            for s in (2 * p, 2 * p + 1):
                for q in range(NQ):
                    xq, invr, aN = quarter(s, q)
                    if pending is not None:
                        (pp, work) = pending
                        aggregate(work, *pend_pair[pp])
                        if work[0] % 2 == 1 and work[1] == NQ - 1:
                            tail_pair(pp, *pend_pair[pp])
                    pending = (p, (s, q, xq, invr, aN))
        (pp, work) = pending
        aggregate(work, *pend_pair[pp])
        tail_pair(pp, *pend_pair[pp])

    return nc


_NC_CACHE = None


def make_in_maps(inputs: dict) -> list:
    x = np.ascontiguousarray(np.asarray(inputs["x"], dtype=np.float32))
    W = np.ascontiguousarray(np.asarray(inputs["W"], dtype=np.float32))
    cent = np.ascontiguousarray(
        np.asarray(inputs["centroids"], dtype=np.float32))

    np8 = mybir.dt.np(FP8)
    # Wpk[p, hc, j2, k] = W[k, 256*hc + 2*p + j2]
    Wr = W.reshape(K, 4, 128, 2)                  # [k, hc, p, j2]
    Wpk = np.ascontiguousarray(
        Wr.transpose(2, 1, 3, 0)).astype(np8)     # [p, hc, j2, k]
    ident = np.eye(K, dtype=np.float32)

    ind2 = np.zeros((2, 128), dtype=np.float32)
    ind2[0, 0:64] = 1.0
    ind2[1, 64:128] = 1.0
    indK = np.zeros((128, 2), dtype=np.float32)
    indK[0:64, 0] = 1.0
    indK[64:128, 1] = 1.0

    return [
        {
            "x": np.ascontiguousarray(
                x[B_PER_CORE * c:B_PER_CORE * (c + 1)]),
            "Wpk": Wpk,
            "centroids": cent,
            "ident": ident,
            "ind2": ind2,
            "indK": indK,
        }
        for c in range(N_CORES)
    ]


def kernel(**inputs: np.ndarray) -> np.ndarray:
    global _NC_CACHE
    _apply_patch()
    from concourse.bass_utils import run_bass_kernel_spmd

    if _NC_CACHE is None:
        _NC_CACHE = build_nc()
    nc = _NC_CACHE

    in_maps = make_in_maps(inputs)
    res = run_bass_kernel_spmd(nc, in_maps, core_ids=list(range(N_CORES)))
    return np.concatenate([r["out"] for r in res.results], axis=0)
